# revision 12
# baseline (speedup 1.0000x reference)
"""Grouped Query Attention on 8 TRN2 NeuronCores.

Sharding: batch x s_q-quarter (core c -> batch c//4, query rows
[512*(c%4), 512*(c%4+1))). Each core computes the Q projection for its
512 query rows, attention for all 16 heads over its query rows, and the
output projection for a disjoint [512, 2048] slice of the output.

The KV projection is sharded: each core projects K^T and V only for its
OWN sequence quarter (= chunk 0 of its rotated x), packs them into a
1 MB DRAM buffer, and a 4-core AllGather per batch assembles the full
K^T/V in canonical sequence order while the tensor engine runs the Q
projection. Attention consumes the gathered K/V (s_k order is
permutation-invariant; K and V share the canonical order).

Other structure (v2):
- All matmul inputs bf16; PSUM accumulation f32.
- x chunk / Q^T / K^T / V / attn outputs are SBUF-resident.
- V is projected directly in [s, d] orientation (lhsT = x^T s-tile) so
  phase 2 needs no PE transposes.
- Scores land in [P, 2, 512] PSUM tiles so each ACT exp instruction
  covers 1024 columns.
- Per-head A-pass (scores+exp) / B-pass (attnV+denominator) software
  pipeline keeps the tensor engine dense (full p-state clock).
- Normalization: ones-matmul denominator -> DVE reciprocal -> GPSIMD
  partition_broadcast -> DVE multiply.
- Bulk weight loads ride the ACT-engine DMA queue so the SP queue only
  carries the latency-critical stream (x chunk, per-head Q weights).
- 1/sqrt(128) folded into Wq on host.
"""

import numpy as np

E = 2048
S = 2048
P = 128
H = 16
G = 4
SQ = 512          # query rows per core
EB = E // P       # 16 e-blocks (contraction tiles)
KV_N = 2 * E // G  # 1024
NCORES = 8

_NC = None
TRACE = False
LAST_RESULT = None


def _build():
    import concourse.bacc as bacc
    import concourse.mybir as mybir
    import concourse.tile as tile

    f32 = mybir.dt.float32
    bf16 = mybir.dt.bfloat16
    EXP = mybir.ActivationFunctionType.Exp
    IDENT = mybir.ActivationFunctionType.Identity

    nc = bacc.Bacc("TRN2", target_bir_lowering=False, debug=False,
                   num_devices=NCORES)

    # host layouts:
    #   xt:  x^T rotated chunk 0 (this core's quarter), [hd, eb, s_own]
    #   wq:  [head, p, eb, p] (1/sqrt(d) folded)
    #   wkv: [p, eb, 1024] with columns [K0 K1 K2 K3 V0 V1 V2 V3]
    #   wo:  [p, eb, e]
    xt = nc.declare_dram_parameter("xt", [P, EB, SQ], bf16, isOutput=False).ap()
    wq = nc.declare_dram_parameter("wq", [H, P, EB, P], bf16, isOutput=False).ap()
    wkv = nc.declare_dram_parameter("wkv", [P, EB, KV_N], bf16, isOutput=False).ap()
    wo = nc.declare_dram_parameter("wo", [P, EB, E], bf16, isOutput=False).ap()
    bq = nc.declare_dram_parameter("bq", [P, H], f32, isOutput=False).ap()
    bkvk = nc.declare_dram_parameter("bkvk", [P, 4], f32, isOutput=False).ap()
    bkvv = nc.declare_dram_parameter("bkvv", [1, 512], bf16, isOutput=False).ap()
    bo = nc.declare_dram_parameter("bo", [1, E], bf16, isOutput=False).ap()
    out = nc.declare_dram_parameter("out", [SQ, E], f32, isOutput=True).ap()

    RG = [[0, 1, 2, 3], [4, 5, 6, 7]]

    with tile.TileContext(nc) as tc:
        with tc.tile_pool(name="consts", bufs=1) as cp, \
             tc.tile_pool(name="qtsp", bufs=1) as qtsp, \
             tc.tile_pool(name="kvp", bufs=1) as kvp, \
             tc.tile_pool(name="otp", bufs=1) as otp, \
             tc.tile_pool(name="dram", bufs=1, space="DRAM") as dp:
            onec = cp.tile([P, 1], bf16, tag="onec")
            nc.vector.memset(onec, 1.0)
            oner = cp.tile([1, P], bf16, tag="oner")
            nc.vector.memset(oner, 1.0)
            # consts ride the ACT queue so the SP queue starts on x at once
            bq_s = cp.tile([P, H], f32, tag="bqs")
            nc.scalar.dma_start(bq_s, bq)
            bkvk_s = cp.tile([P, 4], f32, tag="bkvks")
            nc.scalar.dma_start(bkvk_s, bkvk)
            bkvv_s = cp.tile([1, 512], bf16, tag="bkvvs")
            nc.scalar.dma_start(bkvv_s, bkvv)
            bo_s = cp.tile([1, E], bf16, tag="bos")
            nc.scalar.dma_start(bo_s, bo)

            qts = qtsp.tile([P, H, SQ], bf16, tag="qts")    # Q^T, [hd, head, sq]
            kts = kvp.tile([P, G, S], bf16, tag="kts")      # K^T, [hd, group, sk]
            vgs = kvp.tile([P, EB, 512], bf16, tag="vgs")   # V, [sk, sk_tile, g*128+hd]
            OT = otp.tile([P, H, SQ], bf16, tag="ot")       # attn out, [hd, head, sq]

            # own-quarter KV pack: m 0..3 = K^T groups, m 4..7 = V s-tiles
            kvown = dp.tile([P, 8, 512], bf16, tag="kvown")
            kvall = dp.tile([4, P, 8, 512], bf16, tag="kvall")

            # ---- Phase 1: projections from the SBUF-resident x^T quarter.
            with tc.tile_pool(name="xsp", bufs=1) as xsp, \
                 tc.tile_pool(name="wkvp", bufs=1) as wkvp, \
                 tc.tile_pool(name="kvsg", bufs=1) as kvsg, \
                 tc.tile_pool(name="wqp", bufs=2) as wqp, \
                 tc.tile_pool(name="ps1", bufs=3, space="PSUM") as ps1, \
                 tc.tile_pool(name="ps1b", bufs=3, space="PSUM") as ps1b:
                xs = xsp.tile([P, EB, SQ], bf16, tag="xs")
                # split so the first e-blocks land quickly
                for c4 in range(4):
                    nc.sync.dma_start(xs[:, 4 * c4:4 * (c4 + 1)],
                                      xt[:, 4 * c4:4 * (c4 + 1)])
                wkv_s = wkvp.tile([P, EB, KV_N], bf16, tag="wkvs")
                nc.scalar.dma_start(wkv_s[:, :, 0:512], wkv[:, :, 0:512])
                nc.scalar.dma_start(wkv_s[:, :, 512:KV_N], wkv[:, :, 512:KV_N])
                kvstg = kvsg.tile([P, 8, 512], bf16, tag="kvstg")

                # K^T for all 4 groups over this core's own quarter
                for m in range(G):
                    ps = ps1b.tile([P, 512], f32, tag="ps")
                    for b in range(EB):
                        nc.tensor.matmul(
                            ps, wkv_s[:, b, m * P:(m + 1) * P], xs[:, b],
                            start=(b == 0), stop=(b == EB - 1))
                    nc.scalar.activation(kvstg[:, m], ps, IDENT,
                                         bias=bkvk_s[:, m:m + 1])

                # V in [s, d] orientation for this core's own 4 s-tiles
                for t in range(4):
                    ps = ps1b.tile([P, 512], f32, tag="ps")
                    nc.tensor.matmul(ps, oner, bkvv_s, start=True, stop=False)
                    for b in range(EB):
                        nc.tensor.matmul(
                            ps, xs[:, b, t * P:(t + 1) * P],
                            wkv_s[:, b, 512:KV_N],
                            start=False, stop=(b == EB - 1))
                    nc.vector.tensor_copy(kvstg[:, 4 + t], ps)

                # pack -> DRAM -> AllGather (runs while Q projection continues)
                nc.sync.dma_start(kvown, kvstg)
                nc.gpsimd.collective_compute(
                    "AllGather", mybir.AluOpType.bypass,
                    replica_groups=RG, ins=[kvown[:]], outs=[kvall[:]])
                for g in range(G):
                    nc.sync.dma_start(
                        kts[:, g], kvall[:, :, g].rearrange("c p w -> p c w"))
                nc.sync.dma_start(
                    vgs, kvall[:, :, 4:8].rearrange("c p i w -> p c i w"))

                # Q projection overlaps the collective
                for m in range(H):
                    wqm = wqp.tile([P, EB, P], bf16, tag="wqm")
                    nc.sync.dma_start(wqm, wq[m])
                    ps = ps1.tile([P, SQ], f32, tag="ps")
                    for b in range(EB):
                        nc.tensor.matmul(ps, wqm[:, b], xs[:, b],
                                         start=(b == 0), stop=(b == EB - 1))
                    nc.vector.tensor_scalar_add(qts[:, m], ps, bq_s[:, m:m + 1])

            # ---- Phase 2: attention, A/B software pipeline over heads.
            with tc.tile_pool(name="wop", bufs=2) as wop, \
                 tc.tile_pool(name="eap", bufs=3) as eap, \
                 tc.tile_pool(name="lip", bufs=2) as lip, \
                 tc.tile_pool(name="lbp", bufs=2) as lbp:
                won0 = wop.tile([P, EB, 512], bf16, tag="won")
                nc.scalar.dma_start(won0, wo[:, :, 0:512])  # prefetch phase 3

                with tc.tile_pool(name="pscp", bufs=3, space="PSUM") as pscp, \
                     tc.tile_pool(name="psop", bufs=1, space="PSUM") as psop, \
                     tc.tile_pool(name="pslp", bufs=1, space="PSUM") as pslp:
                    eas = [None, None, None]

                    def a_pass(h):
                        g = h // 4
                        ea = eap.tile([P, EB, SQ], bf16, tag="ea")
                        for j in range(8):
                            ps2 = pscp.tile([P, 2, SQ], f32, tag="ps2")
                            for u in range(2):
                                t = 2 * j + u
                                nc.tensor.matmul(
                                    ps2[:, u], kts[:, g, t * P:(t + 1) * P],
                                    qts[:, h], start=True, stop=True)
                            nc.scalar.activation(ea[:, 2 * j:2 * j + 2], ps2, EXP)
                        eas[h % 3] = ea

                    def b_pass(h):
                        g = h // 4
                        ea = eas[h % 3]
                        pso = psop.tile([P, SQ], f32, tag="pso")
                        psl = pslp.tile([1, SQ], f32, tag="psl")
                        for t in range(EB):
                            nc.tensor.matmul(pso, vgs[:, t, g * P:(g + 1) * P],
                                             ea[:, t], start=(t == 0), stop=(t == EB - 1))
                            nc.tensor.matmul(psl, onec, ea[:, t],
                                             start=(t == 0), stop=(t == EB - 1))
                        li = lip.tile([1, SQ], f32, tag="li")
                        nc.vector.reciprocal(li, psl)
                        lb = lbp.tile([P, SQ], f32, tag="lb")
                        nc.gpsimd.partition_broadcast(lb, li)
                        nc.vector.tensor_mul(OT[:, h], pso, lb)

                    # 2-head lead keeps the scalar engine's exp well ahead of
                    # the B-pass so the tensor engine never waits (and holds
                    # its full p-state clock).
                    a_pass(0)
                    a_pass(1)
                    for h in range(H):
                        if h + 2 < H:
                            a_pass(h + 2)
                        b_pass(h)

                # ---- Phase 3: output projection, contraction over the 16
                # head blocks; bias seeded via a K=1 ones matmul.
                with tc.tile_pool(name="obp", bufs=3) as obp, \
                     tc.tile_pool(name="ps3", bufs=2, space="PSUM") as ps3p:
                    for n in range(4):
                        if n == 0:
                            won = won0
                        else:
                            won = wop.tile([P, EB, 512], bf16, tag="won")
                            nc.scalar.dma_start(won, wo[:, :, 512 * n:512 * (n + 1)])
                        for ms in range(4):
                            ps = ps3p.tile([P, 512], f32, tag="ps")
                            nc.tensor.matmul(
                                ps, oner, bo_s[:, 512 * n:512 * (n + 1)],
                                start=True, stop=False)
                            for k in range(EB):
                                nc.tensor.matmul(
                                    ps, OT[:, k, ms * P:(ms + 1) * P],
                                    won[:, k],
                                    start=False, stop=(k == EB - 1))
                            ob = obp.tile([P, 512], f32, tag="ob")
                            nc.vector.tensor_copy(ob, ps)
                            nc.sync.dma_start(
                                out[ms * P:(ms + 1) * P, 512 * n:512 * (n + 1)], ob)

    nc.compile()
    return nc


def _get_nc():
    global _NC
    if _NC is None:
        _NC = _build()
    return _NC


def kernel(x, Wq, bq, Wkv, bkv, Wo, bo):
    from concourse.bass_utils import run_bass_kernel_spmd
    import ml_dtypes
    global LAST_RESULT

    bf = ml_dtypes.bfloat16
    x = np.asarray(x, np.float32)
    Wq = np.asarray(Wq, np.float32)
    bq = np.asarray(bq, np.float32)
    Wkv = np.asarray(Wkv, np.float32)
    bkv = np.asarray(bkv, np.float32)
    Wo = np.asarray(Wo, np.float32)
    bo = np.asarray(bo, np.float32)

    nc = _get_nc()
    sc = 1.0 / np.sqrt(E // H)
    wq_h = np.ascontiguousarray(
        (Wq * sc).reshape(EB, P, H, P).transpose(2, 1, 0, 3)).astype(bf)
    kcols = np.concatenate([Wkv[:, 256 * g:256 * g + 128] for g in range(G)], axis=1)
    vcols = np.concatenate([Wkv[:, 256 * g + 128:256 * g + 256] for g in range(G)], axis=1)
    wkv_re = np.concatenate([kcols, vcols], axis=1)  # [E, 1024]
    wkv_h = np.ascontiguousarray(wkv_re.reshape(EB, P, KV_N).transpose(1, 0, 2)).astype(bf)
    wo_h = np.ascontiguousarray(Wo.reshape(EB, P, E).transpose(1, 0, 2)).astype(bf)
    bq_h = np.ascontiguousarray((bq * sc).reshape(H, P).T).astype(np.float32)
    bkv_k = np.stack([bkv[256 * g:256 * g + 128] for g in range(G)], axis=1)
    bkv_v = np.concatenate([bkv[256 * g + 128:256 * g + 256] for g in range(G)])
    bkvk_h = np.ascontiguousarray(bkv_k).astype(np.float32)
    bkvv_h = np.ascontiguousarray(bkv_v.reshape(1, 512)).astype(bf)
    bo_h = np.ascontiguousarray(bo.reshape(1, E)).astype(bf)

    in_maps = []
    for c in range(NCORES):
        b, q = divmod(c, 4)
        xq = x[b, 512 * q:512 * (q + 1), :].T  # [e, s_own] — own quarter only
        xt_h = np.ascontiguousarray(
            xq.reshape(EB, P, SQ).transpose(1, 0, 2)).astype(bf)
        in_maps.append({"xt": xt_h, "wq": wq_h, "wkv": wkv_h, "wo": wo_h,
                        "bq": bq_h, "bkvk": bkvk_h, "bkvv": bkvv_h, "bo": bo_h})

    res = run_bass_kernel_spmd(nc, in_maps, core_ids=list(range(NCORES)),
                               trace=TRACE)
    LAST_RESULT = res

    outf = np.empty((2, S, E), np.float32)
    for c in range(NCORES):
        b, q = divmod(c, 4)
        outf[b, 512 * q:512 * (q + 1), :] = res.results[c]["out"]
    return outf


# revision 14
# speedup vs baseline: 1.0730x; 1.0730x over previous
"""Grouped Query Attention on 8 TRN2 NeuronCores.

Sharding: batch x s_q-quarter (core c -> batch c//4, query rows
[512*(c%4), 512*(c%4+1))). Each core computes the Q projection for its
512 query rows, attention for all 16 heads over its query rows, and the
output projection for a disjoint [512, 2048] slice of the output.

The KV projection is sharded: each core projects K^T and V only for its
OWN sequence quarter (= chunk 0 of its rotated x), packs them into a
1 MB DRAM buffer, and a 4-core AllGather per batch assembles the full
K^T/V in canonical sequence order while the tensor engine runs the Q
projection. Attention consumes the gathered K/V (s_k order is
permutation-invariant; K and V share the canonical order).

Other structure (v2):
- All matmul inputs bf16; PSUM accumulation f32.
- x chunk / Q^T / K^T / V / attn outputs are SBUF-resident.
- V is projected directly in [s, d] orientation (lhsT = x^T s-tile) so
  phase 2 needs no PE transposes.
- Scores land in [P, 2, 512] PSUM tiles so each ACT exp instruction
  covers 1024 columns.
- Per-head A-pass (scores+exp) / B-pass (attnV+denominator) software
  pipeline keeps the tensor engine dense (full p-state clock).
- Normalization: ones-matmul denominator -> DVE reciprocal -> GPSIMD
  partition_broadcast -> DVE multiply.
- Bulk weight loads ride the ACT-engine DMA queue so the SP queue only
  carries the latency-critical stream (x chunk, per-head Q weights).
- 1/sqrt(128) folded into Wq on host.
"""

import numpy as np

E = 2048
S = 2048
P = 128
H = 16
G = 4
SQ = 512          # query rows per core
EB = E // P       # 16 e-blocks (contraction tiles)
KV_N = 2 * E // G  # 1024
NCORES = 8

_NC = None
TRACE = False
LAST_RESULT = None


def _build():
    import concourse.bacc as bacc
    import concourse.mybir as mybir
    import concourse.tile as tile

    f32 = mybir.dt.float32
    bf16 = mybir.dt.bfloat16
    EXP = mybir.ActivationFunctionType.Exp
    IDENT = mybir.ActivationFunctionType.Identity

    nc = bacc.Bacc("TRN2", target_bir_lowering=False, debug=False,
                   num_devices=NCORES)

    # host layouts:
    #   xt:  x^T rotated chunk 0 (this core's quarter), [hd, eb, s_own]
    #   wq:  [head, p, eb, p] (1/sqrt(d) folded)
    #   wkv: [p, eb, 1024] with columns [K0 K1 K2 K3 V0 V1 V2 V3]
    #   wo:  [p, eb, e]
    xt = nc.declare_dram_parameter("xt", [P, EB, SQ], bf16, isOutput=False).ap()
    wq = nc.declare_dram_parameter("wq", [H, P, EB, P], bf16, isOutput=False).ap()
    wkv = nc.declare_dram_parameter("wkv", [P, EB, KV_N], bf16, isOutput=False).ap()
    wo = nc.declare_dram_parameter("wo", [P, EB, E], bf16, isOutput=False).ap()
    bq = nc.declare_dram_parameter("bq", [P, H], f32, isOutput=False).ap()
    bkvk = nc.declare_dram_parameter("bkvk", [P, 4], f32, isOutput=False).ap()
    bkvv = nc.declare_dram_parameter("bkvv", [1, 512], bf16, isOutput=False).ap()
    bo = nc.declare_dram_parameter("bo", [1, E], bf16, isOutput=False).ap()
    out = nc.declare_dram_parameter("out", [SQ, E], f32, isOutput=True).ap()

    RG = [[0, 1, 2, 3], [4, 5, 6, 7]]

    with tile.TileContext(nc) as tc:
        with tc.tile_pool(name="consts", bufs=1) as cp, \
             tc.tile_pool(name="qtsp", bufs=1) as qtsp, \
             tc.tile_pool(name="kvp", bufs=1) as kvp, \
             tc.tile_pool(name="otp", bufs=1) as otp, \
             tc.tile_pool(name="dram", bufs=1, space="DRAM") as dp:
            onec = cp.tile([P, 1], bf16, tag="onec")
            nc.vector.memset(onec, 1.0)
            oner = cp.tile([1, P], bf16, tag="oner")
            nc.vector.memset(oner, 1.0)
            # consts ride the ACT queue so the SP queue starts on x at once
            bq_s = cp.tile([P, H], f32, tag="bqs")
            nc.scalar.dma_start(bq_s, bq)
            bkvk_s = cp.tile([P, 4], f32, tag="bkvks")
            nc.scalar.dma_start(bkvk_s, bkvk)
            bkvv_s = cp.tile([1, 512], bf16, tag="bkvvs")
            nc.scalar.dma_start(bkvv_s, bkvv)
            bo_s = cp.tile([1, E], bf16, tag="bos")
            nc.scalar.dma_start(bo_s, bo)

            qts = qtsp.tile([P, H, SQ], bf16, tag="qts")    # Q^T, [hd, head, sq]
            kts = kvp.tile([P, G, S], bf16, tag="kts")      # K^T, [hd, group, sk]
            vgs = kvp.tile([P, EB, 512], bf16, tag="vgs")   # V, [sk, sk_tile, g*128+hd]
            OT = otp.tile([P, H, SQ], bf16, tag="ot")       # attn out, [hd, head, sq]

            # own-quarter KV pack: m 0..3 = K^T groups, m 4..7 = V s-tiles
            kvown = dp.tile([P, 8, 512], bf16, tag="kvown")
            kvall = dp.tile([4, P, 8, 512], bf16, tag="kvall")

            # ---- Phase 1: projections from the SBUF-resident x^T quarter.
            with tc.tile_pool(name="xsp", bufs=1) as xsp, \
                 tc.tile_pool(name="wkvp", bufs=1) as wkvp, \
                 tc.tile_pool(name="kvsg", bufs=1) as kvsg, \
                 tc.tile_pool(name="wqp", bufs=2) as wqp, \
                 tc.tile_pool(name="ps1", bufs=3, space="PSUM") as ps1, \
                 tc.tile_pool(name="ps1b", bufs=3, space="PSUM") as ps1b:
                xs = xsp.tile([P, EB, SQ], bf16, tag="xs")
                # split so the first e-blocks land quickly
                for c4 in range(4):
                    nc.sync.dma_start(xs[:, 4 * c4:4 * (c4 + 1)],
                                      xt[:, 4 * c4:4 * (c4 + 1)])
                wkv_s = wkvp.tile([P, EB, KV_N], bf16, tag="wkvs")
                nc.scalar.dma_start(wkv_s[:, :, 0:512], wkv[:, :, 0:512])
                nc.scalar.dma_start(wkv_s[:, :, 512:KV_N], wkv[:, :, 512:KV_N])
                kvstg = kvsg.tile([P, 8, 512], bf16, tag="kvstg")

                # K^T for all 4 groups over this core's own quarter
                for m in range(G):
                    ps = ps1b.tile([P, 512], f32, tag="ps")
                    for b in range(EB):
                        nc.tensor.matmul(
                            ps, wkv_s[:, b, m * P:(m + 1) * P], xs[:, b],
                            start=(b == 0), stop=(b == EB - 1))
                    nc.scalar.activation(kvstg[:, m], ps, IDENT,
                                         bias=bkvk_s[:, m:m + 1])

                # V in [s, d] orientation for this core's own 4 s-tiles
                for t in range(4):
                    ps = ps1b.tile([P, 512], f32, tag="ps")
                    nc.tensor.matmul(ps, oner, bkvv_s, start=True, stop=False)
                    for b in range(EB):
                        nc.tensor.matmul(
                            ps, xs[:, b, t * P:(t + 1) * P],
                            wkv_s[:, b, 512:KV_N],
                            start=False, stop=(b == EB - 1))
                    nc.vector.tensor_copy(kvstg[:, 4 + t], ps)

                # pack -> DRAM -> AllGather (runs while Q projection continues)
                nc.sync.dma_start(kvown, kvstg)
                nc.gpsimd.collective_compute(
                    "AllGather", mybir.AluOpType.bypass,
                    replica_groups=RG, ins=[kvown[:]], outs=[kvall[:]])
                for g in range(G):
                    nc.sync.dma_start(
                        kts[:, g], kvall[:, :, g].rearrange("c p w -> p c w"))
                nc.sync.dma_start(
                    vgs, kvall[:, :, 4:8].rearrange("c p i w -> p c i w"))

                # Q projection overlaps the collective
                for m in range(H):
                    wqm = wqp.tile([P, EB, P], bf16, tag="wqm")
                    nc.sync.dma_start(wqm, wq[m])
                    ps = ps1.tile([P, SQ], f32, tag="ps")
                    for b in range(EB):
                        nc.tensor.matmul(ps, wqm[:, b], xs[:, b],
                                         start=(b == 0), stop=(b == EB - 1))
                    nc.vector.tensor_scalar_add(qts[:, m], ps, bq_s[:, m:m + 1])

            # ---- Phase 2: attention, A/B software pipeline over heads.
            with tc.tile_pool(name="wop", bufs=2) as wop, \
                 tc.tile_pool(name="eap", bufs=3) as eap, \
                 tc.tile_pool(name="lip", bufs=2) as lip, \
                 tc.tile_pool(name="lbp", bufs=2) as lbp:
                won0 = wop.tile([P, EB, 512], bf16, tag="won")
                nc.scalar.dma_start(won0, wo[:, :, 0:512])  # prefetch phase 3

                with tc.tile_pool(name="pscp", bufs=2, space="PSUM") as pscp, \
                     tc.tile_pool(name="psop", bufs=2, space="PSUM") as psop, \
                     tc.tile_pool(name="pslp", bufs=2, space="PSUM") as pslp:
                    eas = [None, None, None]

                    def a_pass(h):
                        g = h // 4
                        ea = eap.tile([P, EB, SQ], bf16, tag="ea")
                        for j in range(8):
                            ps2 = pscp.tile([P, 2, SQ], f32, tag="ps2")
                            for u in range(2):
                                t = 2 * j + u
                                nc.tensor.matmul(
                                    ps2[:, u], kts[:, g, t * P:(t + 1) * P],
                                    qts[:, h], start=True, stop=True)
                            nc.scalar.activation(ea[:, 2 * j:2 * j + 2], ps2, EXP)
                        eas[h % 3] = ea

                    def b_pass(h):
                        g = h // 4
                        ea = eas[h % 3]
                        pso = psop.tile([P, SQ], f32, tag="pso")
                        psl = pslp.tile([1, SQ], f32, tag="psl")
                        for t in range(EB):
                            nc.tensor.matmul(pso, vgs[:, t, g * P:(g + 1) * P],
                                             ea[:, t], start=(t == 0), stop=(t == EB - 1))
                            nc.tensor.matmul(psl, onec, ea[:, t],
                                             start=(t == 0), stop=(t == EB - 1))
                        li = lip.tile([1, SQ], f32, tag="li")
                        nc.vector.reciprocal(li, psl)
                        lb = lbp.tile([P, SQ], f32, tag="lb")
                        nc.gpsimd.partition_broadcast(lb, li)
                        nc.vector.tensor_mul(OT[:, h], pso, lb)

                    a_pass(0)
                    for h in range(H):
                        if h + 1 < H:
                            a_pass(h + 1)
                        b_pass(h)

                # ---- Phase 3: output projection, contraction over the 16
                # head blocks; bias seeded via a K=1 ones matmul.
                with tc.tile_pool(name="obp", bufs=3) as obp, \
                     tc.tile_pool(name="ps3", bufs=2, space="PSUM") as ps3p:
                    for n in range(4):
                        if n == 0:
                            won = won0
                        else:
                            won = wop.tile([P, EB, 512], bf16, tag="won")
                            nc.scalar.dma_start(won, wo[:, :, 512 * n:512 * (n + 1)])
                        for ms in range(4):
                            ps = ps3p.tile([P, 512], f32, tag="ps")
                            nc.tensor.matmul(
                                ps, oner, bo_s[:, 512 * n:512 * (n + 1)],
                                start=True, stop=False)
                            for k in range(EB):
                                nc.tensor.matmul(
                                    ps, OT[:, k, ms * P:(ms + 1) * P],
                                    won[:, k],
                                    start=False, stop=(k == EB - 1))
                            ob = obp.tile([P, 512], f32, tag="ob")
                            nc.vector.tensor_copy(ob, ps)
                            nc.sync.dma_start(
                                out[ms * P:(ms + 1) * P, 512 * n:512 * (n + 1)], ob)

    nc.compile()
    return nc


def _get_nc():
    global _NC
    if _NC is None:
        _NC = _build()
    return _NC


def kernel(x, Wq, bq, Wkv, bkv, Wo, bo):
    from concourse.bass_utils import run_bass_kernel_spmd
    import ml_dtypes
    global LAST_RESULT

    bf = ml_dtypes.bfloat16
    x = np.asarray(x, np.float32)
    Wq = np.asarray(Wq, np.float32)
    bq = np.asarray(bq, np.float32)
    Wkv = np.asarray(Wkv, np.float32)
    bkv = np.asarray(bkv, np.float32)
    Wo = np.asarray(Wo, np.float32)
    bo = np.asarray(bo, np.float32)

    nc = _get_nc()
    sc = 1.0 / np.sqrt(E // H)
    wq_h = np.ascontiguousarray(
        (Wq * sc).reshape(EB, P, H, P).transpose(2, 1, 0, 3)).astype(bf)
    kcols = np.concatenate([Wkv[:, 256 * g:256 * g + 128] for g in range(G)], axis=1)
    vcols = np.concatenate([Wkv[:, 256 * g + 128:256 * g + 256] for g in range(G)], axis=1)
    wkv_re = np.concatenate([kcols, vcols], axis=1)  # [E, 1024]
    wkv_h = np.ascontiguousarray(wkv_re.reshape(EB, P, KV_N).transpose(1, 0, 2)).astype(bf)
    wo_h = np.ascontiguousarray(Wo.reshape(EB, P, E).transpose(1, 0, 2)).astype(bf)
    bq_h = np.ascontiguousarray((bq * sc).reshape(H, P).T).astype(np.float32)
    bkv_k = np.stack([bkv[256 * g:256 * g + 128] for g in range(G)], axis=1)
    bkv_v = np.concatenate([bkv[256 * g + 128:256 * g + 256] for g in range(G)])
    bkvk_h = np.ascontiguousarray(bkv_k).astype(np.float32)
    bkvv_h = np.ascontiguousarray(bkv_v.reshape(1, 512)).astype(bf)
    bo_h = np.ascontiguousarray(bo.reshape(1, E)).astype(bf)

    in_maps = []
    for c in range(NCORES):
        b, q = divmod(c, 4)
        xq = x[b, 512 * q:512 * (q + 1), :].T  # [e, s_own] — own quarter only
        xt_h = np.ascontiguousarray(
            xq.reshape(EB, P, SQ).transpose(1, 0, 2)).astype(bf)
        in_maps.append({"xt": xt_h, "wq": wq_h, "wkv": wkv_h, "wo": wo_h,
                        "bq": bq_h, "bkvk": bkvk_h, "bkvv": bkvv_h, "bo": bo_h})

    res = run_bass_kernel_spmd(nc, in_maps, core_ids=list(range(NCORES)),
                               trace=TRACE)
    LAST_RESULT = res

    outf = np.empty((2, S, E), np.float32)
    for c in range(NCORES):
        b, q = divmod(c, 4)
        outf[b, 512 * q:512 * (q + 1), :] = res.results[c]["out"]
    return outf


# revision 16
# speedup vs baseline: 1.1791x; 1.0989x over previous
"""Grouped Query Attention on 8 TRN2 NeuronCores.

Sharding: batch x s_q-quarter (core c -> batch c//4, query rows
[512*(c%4), 512*(c%4+1))). Each core computes the Q projection for its
512 query rows, attention for all 16 heads over its query rows, and the
output projection for a disjoint [512, 2048] slice of the output.

The KV projection is sharded: each core projects K^T and V only for its
OWN sequence quarter (= chunk 0 of its rotated x), packs them into a
1 MB DRAM buffer, and a 4-core AllGather per batch assembles the full
K^T/V in canonical sequence order while the tensor engine runs the Q
projection. Attention consumes the gathered K/V (s_k order is
permutation-invariant; K and V share the canonical order).

Other structure (v2):
- All matmul inputs bf16; PSUM accumulation f32.
- x chunk / Q^T / K^T / V / attn outputs are SBUF-resident.
- V is projected directly in [s, d] orientation (lhsT = x^T s-tile) so
  phase 2 needs no PE transposes.
- Scores land in [P, 2, 512] PSUM tiles so each ACT exp instruction
  covers 1024 columns.
- Per-head A-pass (scores+exp) / B-pass (attnV+denominator) software
  pipeline keeps the tensor engine dense (full p-state clock).
- Normalization: ones-matmul denominator -> DVE reciprocal -> GPSIMD
  partition_broadcast -> DVE multiply.
- Bulk weight loads ride the ACT-engine DMA queue so the SP queue only
  carries the latency-critical stream (x chunk, per-head Q weights).
- 1/sqrt(128) folded into Wq on host.
"""

import numpy as np

E = 2048
S = 2048
P = 128
H = 16
G = 4
SQ = 512          # query rows per core
EB = E // P       # 16 e-blocks (contraction tiles)
KV_N = 2 * E // G  # 1024
NCORES = 8

_NC = None
TRACE = False
LAST_RESULT = None


def _build():
    import concourse.bacc as bacc
    import concourse.mybir as mybir
    import concourse.tile as tile

    f32 = mybir.dt.float32
    bf16 = mybir.dt.bfloat16
    EXP = mybir.ActivationFunctionType.Exp
    IDENT = mybir.ActivationFunctionType.Identity

    nc = bacc.Bacc("TRN2", target_bir_lowering=False, debug=False,
                   num_devices=NCORES)

    # host layouts:
    #   xt:  x^T rotated chunk 0 (this core's quarter), [hd, eb, s_own]
    #   wq:  [head, p, eb, p] (1/sqrt(d) folded)
    #   wkv: [p, eb, 1024] with columns [K0 K1 K2 K3 V0 V1 V2 V3]
    #   wo:  [p, eb, e]
    xt = nc.declare_dram_parameter("xt", [P, EB, SQ], bf16, isOutput=False).ap()
    wq = nc.declare_dram_parameter("wq", [H, P, EB, P], bf16, isOutput=False).ap()
    wkv = nc.declare_dram_parameter("wkv", [P, EB, KV_N], bf16, isOutput=False).ap()
    wo = nc.declare_dram_parameter("wo", [P, EB, E], bf16, isOutput=False).ap()
    bq = nc.declare_dram_parameter("bq", [P, H], f32, isOutput=False).ap()
    bkvk = nc.declare_dram_parameter("bkvk", [P, 4], f32, isOutput=False).ap()
    bkvv = nc.declare_dram_parameter("bkvv", [1, 512], bf16, isOutput=False).ap()
    bo = nc.declare_dram_parameter("bo", [1, E], bf16, isOutput=False).ap()
    out = nc.declare_dram_parameter("out", [SQ, E], f32, isOutput=True).ap()

    RG = [[0, 1, 2, 3], [4, 5, 6, 7]]

    with tile.TileContext(nc) as tc:
        with tc.tile_pool(name="consts", bufs=1) as cp, \
             tc.tile_pool(name="qtsp", bufs=1) as qtsp, \
             tc.tile_pool(name="kvp", bufs=1) as kvp, \
             tc.tile_pool(name="otp", bufs=1) as otp, \
             tc.tile_pool(name="dram", bufs=1, space="DRAM") as dp:
            onec = cp.tile([P, 1], bf16, tag="onec")
            nc.vector.memset(onec, 1.0)
            oner = cp.tile([1, P], bf16, tag="oner")
            nc.vector.memset(oner, 1.0)
            # consts ride the ACT queue so the SP queue starts on x at once
            bq_s = cp.tile([P, H], f32, tag="bqs")
            nc.scalar.dma_start(bq_s, bq)
            bkvk_s = cp.tile([P, 4], f32, tag="bkvks")
            nc.scalar.dma_start(bkvk_s, bkvk)
            bkvv_s = cp.tile([1, 512], bf16, tag="bkvvs")
            nc.scalar.dma_start(bkvv_s, bkvv)
            bo_s = cp.tile([1, E], bf16, tag="bos")
            nc.scalar.dma_start(bo_s, bo)

            qts = qtsp.tile([P, H, SQ], bf16, tag="qts")    # Q^T, [hd, head, sq]
            kts = kvp.tile([P, G, S], bf16, tag="kts")      # K^T, [hd, group, sk]
            vgs = kvp.tile([P, EB, 512], bf16, tag="vgs")   # V, [sk, sk_tile, g*128+hd]
            OT = otp.tile([P, H, SQ], bf16, tag="ot")       # attn out, [hd, head, sq]

            # own-quarter KV pack: m 0..3 = K^T groups, m 4..7 = V s-tiles
            kvown = dp.tile([P, 8, 512], bf16, tag="kvown")
            kvall = dp.tile([4, P, 8, 512], bf16, tag="kvall")

            # ---- Phase 1: projections from the SBUF-resident x^T quarter.
            with tc.tile_pool(name="xsp", bufs=1) as xsp, \
                 tc.tile_pool(name="wkvp", bufs=1) as wkvp, \
                 tc.tile_pool(name="kvsg", bufs=1) as kvsg, \
                 tc.tile_pool(name="wqp", bufs=2) as wqp, \
                 tc.tile_pool(name="ps1", bufs=3, space="PSUM") as ps1, \
                 tc.tile_pool(name="ps1b", bufs=3, space="PSUM") as ps1b:
                xs = xsp.tile([P, EB, SQ], bf16, tag="xs")
                # split so the first e-blocks land quickly
                for c4 in range(4):
                    nc.sync.dma_start(xs[:, 4 * c4:4 * (c4 + 1)],
                                      xt[:, 4 * c4:4 * (c4 + 1)])
                wkv_s = wkvp.tile([P, EB, KV_N], bf16, tag="wkvs")
                nc.scalar.dma_start(wkv_s[:, :, 0:512], wkv[:, :, 0:512])
                nc.scalar.dma_start(wkv_s[:, :, 512:KV_N], wkv[:, :, 512:KV_N])
                kvstg = kvsg.tile([P, 8, 512], bf16, tag="kvstg")

                def q_head(m):
                    wqm = wqp.tile([P, EB, P], bf16, tag="wqm")
                    nc.sync.dma_start(wqm, wq[m])
                    ps = ps1.tile([P, SQ], f32, tag="ps")
                    for b in range(EB):
                        nc.tensor.matmul(ps, wqm[:, b], xs[:, b],
                                         start=(b == 0), stop=(b == EB - 1))
                    nc.vector.tensor_scalar_add(qts[:, m], ps, bq_s[:, m:m + 1])

                # two Q heads first so the PE starts immediately
                q_head(0)
                q_head(1)

                # K^T for all 4 groups over this core's own quarter
                for m in range(G):
                    ps = ps1b.tile([P, 512], f32, tag="ps")
                    for b in range(EB):
                        nc.tensor.matmul(
                            ps, wkv_s[:, b, m * P:(m + 1) * P], xs[:, b],
                            start=(b == 0), stop=(b == EB - 1))
                    nc.scalar.activation(kvstg[:, m], ps, IDENT,
                                         bias=bkvk_s[:, m:m + 1])

                # V in [s, d] orientation for this core's own 4 s-tiles
                for t in range(4):
                    ps = ps1b.tile([P, 512], f32, tag="ps")
                    nc.tensor.matmul(ps, oner, bkvv_s, start=True, stop=False)
                    for b in range(EB):
                        nc.tensor.matmul(
                            ps, xs[:, b, t * P:(t + 1) * P],
                            wkv_s[:, b, 512:KV_N],
                            start=False, stop=(b == EB - 1))
                    nc.vector.tensor_copy(kvstg[:, 4 + t], ps)

                # pack -> DRAM -> AllGather (runs while Q projection continues)
                nc.sync.dma_start(kvown, kvstg)
                nc.gpsimd.collective_compute(
                    "AllGather", mybir.AluOpType.bypass,
                    replica_groups=RG, ins=[kvown[:]], outs=[kvall[:]])
                for g in range(G):
                    nc.sync.dma_start(
                        kts[:, g], kvall[:, :, g].rearrange("c p w -> p c w"))
                nc.sync.dma_start(
                    vgs, kvall[:, :, 4:8].rearrange("c p i w -> p c i w"))

                # remaining Q heads overlap the collective
                for m in range(2, H):
                    q_head(m)

            # ---- Phase 2: attention, A/B software pipeline over heads.
            with tc.tile_pool(name="wop", bufs=2) as wop, \
                 tc.tile_pool(name="eap", bufs=3) as eap, \
                 tc.tile_pool(name="lip", bufs=2) as lip, \
                 tc.tile_pool(name="lbp", bufs=2) as lbp:
                won0 = wop.tile([P, EB, 512], bf16, tag="won")
                nc.scalar.dma_start(won0, wo[:, :, 0:512])  # prefetch phase 3

                with tc.tile_pool(name="pscp", bufs=2, space="PSUM") as pscp, \
                     tc.tile_pool(name="psop", bufs=2, space="PSUM") as psop, \
                     tc.tile_pool(name="pslp", bufs=2, space="PSUM") as pslp:
                    eas = [None, None, None]

                    def a_pass(h):
                        g = h // 4
                        ea = eap.tile([P, EB, SQ], bf16, tag="ea")
                        for j in range(8):
                            ps2 = pscp.tile([P, 2, SQ], f32, tag="ps2")
                            for u in range(2):
                                t = 2 * j + u
                                nc.tensor.matmul(
                                    ps2[:, u], kts[:, g, t * P:(t + 1) * P],
                                    qts[:, h], start=True, stop=True)
                            nc.scalar.activation(ea[:, 2 * j:2 * j + 2], ps2, EXP)
                        eas[h % 3] = ea

                    def b_pass(h):
                        g = h // 4
                        ea = eas[h % 3]
                        pso = psop.tile([P, SQ], f32, tag="pso")
                        psl = pslp.tile([1, SQ], f32, tag="psl")
                        for t in range(EB):
                            nc.tensor.matmul(pso, vgs[:, t, g * P:(g + 1) * P],
                                             ea[:, t], start=(t == 0), stop=(t == EB - 1))
                            nc.tensor.matmul(psl, onec, ea[:, t],
                                             start=(t == 0), stop=(t == EB - 1))
                        li = lip.tile([1, SQ], f32, tag="li")
                        nc.vector.reciprocal(li, psl)
                        lb = lbp.tile([P, SQ], f32, tag="lb")
                        nc.gpsimd.partition_broadcast(lb, li)
                        nc.vector.tensor_mul(OT[:, h], pso, lb)

                    a_pass(0)
                    for h in range(H):
                        if h + 1 < H:
                            a_pass(h + 1)
                        b_pass(h)

                # ---- Phase 3: output projection, contraction over the 16
                # head blocks; bias seeded via a K=1 ones matmul.
                with tc.tile_pool(name="obp", bufs=3) as obp, \
                     tc.tile_pool(name="ps3", bufs=2, space="PSUM") as ps3p:
                    for n in range(4):
                        if n == 0:
                            won = won0
                        else:
                            won = wop.tile([P, EB, 512], bf16, tag="won")
                            nc.scalar.dma_start(won, wo[:, :, 512 * n:512 * (n + 1)])
                        for ms in range(4):
                            ps = ps3p.tile([P, 512], f32, tag="ps")
                            nc.tensor.matmul(
                                ps, oner, bo_s[:, 512 * n:512 * (n + 1)],
                                start=True, stop=False)
                            for k in range(EB):
                                nc.tensor.matmul(
                                    ps, OT[:, k, ms * P:(ms + 1) * P],
                                    won[:, k],
                                    start=False, stop=(k == EB - 1))
                            ob = obp.tile([P, 512], f32, tag="ob")
                            nc.vector.tensor_copy(ob, ps)
                            nc.sync.dma_start(
                                out[ms * P:(ms + 1) * P, 512 * n:512 * (n + 1)], ob)

    nc.compile()
    return nc


def _get_nc():
    global _NC
    if _NC is None:
        _NC = _build()
    return _NC


def kernel(x, Wq, bq, Wkv, bkv, Wo, bo):
    from concourse.bass_utils import run_bass_kernel_spmd
    import ml_dtypes
    global LAST_RESULT

    bf = ml_dtypes.bfloat16
    x = np.asarray(x, np.float32)
    Wq = np.asarray(Wq, np.float32)
    bq = np.asarray(bq, np.float32)
    Wkv = np.asarray(Wkv, np.float32)
    bkv = np.asarray(bkv, np.float32)
    Wo = np.asarray(Wo, np.float32)
    bo = np.asarray(bo, np.float32)

    nc = _get_nc()
    sc = 1.0 / np.sqrt(E // H)
    wq_h = np.ascontiguousarray(
        (Wq * sc).reshape(EB, P, H, P).transpose(2, 1, 0, 3)).astype(bf)
    kcols = np.concatenate([Wkv[:, 256 * g:256 * g + 128] for g in range(G)], axis=1)
    vcols = np.concatenate([Wkv[:, 256 * g + 128:256 * g + 256] for g in range(G)], axis=1)
    wkv_re = np.concatenate([kcols, vcols], axis=1)  # [E, 1024]
    wkv_h = np.ascontiguousarray(wkv_re.reshape(EB, P, KV_N).transpose(1, 0, 2)).astype(bf)
    wo_h = np.ascontiguousarray(Wo.reshape(EB, P, E).transpose(1, 0, 2)).astype(bf)
    bq_h = np.ascontiguousarray((bq * sc).reshape(H, P).T).astype(np.float32)
    bkv_k = np.stack([bkv[256 * g:256 * g + 128] for g in range(G)], axis=1)
    bkv_v = np.concatenate([bkv[256 * g + 128:256 * g + 256] for g in range(G)])
    bkvk_h = np.ascontiguousarray(bkv_k).astype(np.float32)
    bkvv_h = np.ascontiguousarray(bkv_v.reshape(1, 512)).astype(bf)
    bo_h = np.ascontiguousarray(bo.reshape(1, E)).astype(bf)

    in_maps = []
    for c in range(NCORES):
        b, q = divmod(c, 4)
        xq = x[b, 512 * q:512 * (q + 1), :].T  # [e, s_own] — own quarter only
        xt_h = np.ascontiguousarray(
            xq.reshape(EB, P, SQ).transpose(1, 0, 2)).astype(bf)
        in_maps.append({"xt": xt_h, "wq": wq_h, "wkv": wkv_h, "wo": wo_h,
                        "bq": bq_h, "bkvk": bkvk_h, "bkvv": bkvv_h, "bo": bo_h})

    res = run_bass_kernel_spmd(nc, in_maps, core_ids=list(range(NCORES)),
                               trace=TRACE)
    LAST_RESULT = res

    outf = np.empty((2, S, E), np.float32)
    for c in range(NCORES):
        b, q = divmod(c, 4)
        outf[b, 512 * q:512 * (q + 1), :] = res.results[c]["out"]
    return outf
